# revision 24
# baseline (speedup 1.0000x reference)
"""Trainium2 Bass kernel for causal MHA block (b=4, s=2048, E=1024, 16 heads).

Sharding: tensor-parallel over heads — 2 heads per core across 8 cores.
Each core computes Q^T/K^T (transposed layout, head-packed), V (natural
layout, ones-augmented), block-causal attention with softmax denominators
obtained for free from the ones column, and a partial out-projection over
its 128 embedding dims. Host sums the 8 partials and adds out_b.

bf16 fast path (matmul = 1 cycle/row vs fp32's 4):
  - all matmul operands bf16; PSUM accumulation stays fp32.
  - key-padding mask folded into V: masked keys get v rows and the
    ones-augmentation column zeroed (per-partition tensor_scalar_mul), so
    exp needs no bias and both heads' score tiles share one ACT
    instruction ([128, 2, W] AP over a 2-bank PSUM tile).
  - causal triangle added on the PE: identity @ tri accumulated into the
    diagonal 128-col subregion of the score tile.
  - 1/denom broadcast across partitions via gpsimd partition_broadcast.
  - V projection emits both heads + ones columns in one 130-wide matmul
    group (zero weight column + bias 1.0 produces the ones column).
  - out-proj PSUM->SBUF staging split between Pool and DVE engines.
  - phase A (projections) issue-interleaved with attention one batch
    ahead so ACT/DVE start early and drain before the PE tail.
"""

import sys
from contextlib import ExitStack

import numpy as np

sys.path.insert(0, "/opt/trn_rl_repo")

import concourse.bass as bass  # noqa: E402
import concourse.tile as tile  # noqa: E402
from concourse import bacc  # noqa: E402
from concourse import mybir  # noqa: E402

F32 = mybir.dt.float32
BF16 = mybir.dt.bfloat16
AF = mybir.ActivationFunctionType

NEG = -10000.0
N_CORES = 8


def build_program(B=4, S=2048, io_dt=BF16, rep=1):
    """Build the single-core SPMD program. Returns nc."""
    P = 128
    E = 1024
    ET = E // P            # 8 E-tiles
    RC = 512               # row chunk for projections
    NCH = S // RC          # proj chunks per batch (4)
    NT = S // P            # s-tiles per batch (16)
    TJ = S // 512          # query chunks of 512 per batch (4)
    ROWS = B * S

    nc = bacc.Bacc("TRN2", target_bir_lowering=False, debug=False)

    xT_d = nc.declare_dram_parameter("xT", [E, ROWS], io_dt, isOutput=False)
    wq_d = nc.declare_dram_parameter("wq", [E, P], io_dt, isOutput=False)
    wk_d = nc.declare_dram_parameter("wk", [E, P], io_dt, isOutput=False)
    wv_d = nc.declare_dram_parameter("wv", [E, P], io_dt, isOutput=False)
    bq_d = nc.declare_dram_parameter("bq", [P, 1], F32, isOutput=False)
    bv_d = nc.declare_dram_parameter("bv", [P, 1], F32, isOutput=False)
    ow_d = nc.declare_dram_parameter("ow", [P, E], io_dt, isOutput=False)
    msk_d = nc.declare_dram_parameter("msk", [P, B * NT], F32, isOutput=False)
    tri_d = nc.declare_dram_parameter("tri", [P, P], io_dt, isOutput=False)
    idn_d = nc.declare_dram_parameter("idn", [P, P], io_dt, isOutput=False)
    out_d = nc.declare_dram_parameter("outp", [ROWS, E], io_dt, isOutput=True)

    with ExitStack() as ctx:
        tc = ctx.enter_context(tile.TileContext(nc))
        const = ctx.enter_context(tc.tile_pool(name="const", bufs=1))

        wq_sb = const.tile([P, ET, P], io_dt, tag="wq")
        wk_sb = const.tile([P, ET, P], io_dt, tag="wk")
        wv_sb = const.tile([P, ET, P], io_dt, tag="wv")
        # spread big const loads across engine DMA queues so they transfer
        # in parallel instead of serializing on one queue
        # per-et slices so the first projection matmuls unblock after 32KB
        # instead of a full 256KB weight transfer; spread across queues
        for et in range(ET):
            esl = slice(et * P, (et + 1) * P)
            nc.gpsimd.dma_start(wq_sb[:, et], wq_d[esl, :])
            nc.scalar.dma_start(wk_sb[:, et], wk_d[esl, :])
            nc.gpsimd.dma_start(wv_sb[:, et], wv_d[esl, :])
        ow_sb = const.tile([P, E], io_dt, tag="ow")
        nc.scalar.dma_start(ow_sb[:], ow_d[:])
        bq_sb = const.tile([P, 1], F32, tag="bq")
        nc.gpsimd.dma_start(bq_sb[:], bq_d[:])
        bv_sb = const.tile([P, 1], F32, tag="bv")
        nc.gpsimd.dma_start(bv_sb[:], bv_d[:])
        msk_sb = const.tile([P, B * NT], F32, tag="msk")
        nc.gpsimd.dma_start(msk_sb[:], msk_d[:])
        tri_sb = const.tile([P, P], io_dt, tag="tri")
        nc.gpsimd.dma_start(tri_sb[:], tri_d[:])
        idn_sb = const.tile([P, P], io_dt, tag="idn")
        nc.gpsimd.dma_start(idn_sb[:], idn_d[:])
        ones_sb = const.tile([1, P], io_dt, tag="ones")
        nc.any.memset(ones_sb[:], 1.0)

        # per-batch projection outputs
        qt_sbs = [const.tile([P, S], io_dt, tag=f"qt{b}", name=f"qt{b}") for b in range(B)]
        kt_sbs = [const.tile([P, S], io_dt, tag=f"kt{b}", name=f"kt{b}") for b in range(B)]
        v_sbs = [const.tile([P, NT, 130], io_dt, tag=f"v{b}", name=f"v{b}") for b in range(B)]
        # ones-augmentation columns hold the key-padding mask (1 valid / 0
        # padded) so denominators count only valid keys
        for b in range(B):
            nc.gpsimd.dma_start(v_sbs[b][:, :, 64:65], msk_d[:, b * NT:(b + 1) * NT])
            nc.gpsimd.dma_start(v_sbs[b][:, :, 129:130], msk_d[:, b * NT:(b + 1) * NT])

        xpool = ctx.enter_context(tc.tile_pool(name="xp", bufs=2))
        ppool = ctx.enter_context(tc.tile_pool(name="pt", bufs=4))
        cpool = ctx.enter_context(tc.tile_pool(name="cn", bufs=2))
        spool = ctx.enter_context(tc.tile_pool(name="sm", bufs=2))
        opool = ctx.enter_context(tc.tile_pool(name="ot", bufs=4))
        # PSUM: "s" = 2-bank [128,2,512] tiles shared by phase A groups and
        # attention score pairs (2 bufs = 4 banks); "c" ctx accum (2 banks);
        # "o" out-proj (2 banks).  Total 8 banks.
        psS = ctx.enter_context(tc.tile_pool(name="psS", bufs=2, space="PSUM"))
        psC = ctx.enter_context(tc.tile_pool(name="psC", bufs=2, space="PSUM"))
        psO = ctx.enter_context(tc.tile_pool(name="psO", bufs=2, space="PSUM"))

        def proj_chunk(b, ch):
            """Project rows [b*S + ch*RC, +RC) -> qt/kt/v for batch b."""
            r0 = b * S + ch * RC
            rsb = slice(ch * RC, (ch + 1) * RC)
            xt = xpool.tile([P, ET, RC], io_dt, tag="xt")
            for et in range(ET):
                nc.sync.dma_start(xt[:, et], xT_d[et * P:(et + 1) * P, r0:r0 + RC])
            for w_sb, dst, bias in ((wq_sb, qt_sbs[b], bq_sb), (wk_sb, kt_sbs[b], None)):
                ps = psS.tile([P, 2, RC], F32, tag="s")
                for et in range(ET):
                    nc.tensor.matmul(
                        ps[:, 0], w_sb[:, et, 0:P], xt[:, et],
                        start=(et == 0), stop=(et == ET - 1),
                    )
                if bias is not None:
                    nc.vector.tensor_scalar_add(dst[:, rsb], ps[:, 0], bias[:])
                else:
                    nc.vector.tensor_copy(dst[:, rsb], ps[:, 0])
            # V computed transposed like Q/K (few big matmuls), bias added on
            # the per-partition copy, then PE-transposed into [keys, dims]
            # layout with the key-padding mask folded in on eviction.
            psv = psS.tile([P, 2, RC], F32, tag="s")
            for et in range(ET):
                nc.tensor.matmul(
                    psv[:, 0], wv_sb[:, et], xt[:, et],
                    start=(et == 0), stop=(et == ET - 1),
                )
            vt = xpool.tile([P, RC], io_dt, tag="vt")
            nc.vector.tensor_scalar_add(vt[:], psv[:, 0], bv_sb[:])
            for rt4 in range(RC // P):
                rt = ch * (RC // P) + rt4
                trp = psO.tile([P, P], io_dt, tag="o")
                nc.tensor.transpose(trp[:], vt[:, rt4 * P:(rt4 + 1) * P], idn_sb[:])
                msc = msk_sb[:, b * NT + rt:b * NT + rt + 1]
                nc.vector.tensor_scalar_mul(v_sbs[b][:, rt, 0:64], trp[:, 0:64], msc)
                nc.vector.tensor_scalar_mul(v_sbs[b][:, rt, 65:129], trp[:, 64:128], msc)

        pending = []  # deferred out-proj of the previous chunk

        def attn_chunk(b, j):
            """Attention for query chunk j of batch b; out-proj deferred into
            the next chunk's score loop so the PE never head-of-line-blocks
            on the normalization chain."""
            t0 = j * 512
            nv = 4 * j + 4
            cn = cpool.tile([P, 512], io_dt, tag="cn")
            cps = [psC.tile([65, 512], F32, tag="c", name=f"cps{h}") for h in range(2)]
            for i in range(nv):
                if i == 2 and pending:
                    pending.pop()()
                delta = i * P - t0
                col0 = max(0, delta)
                sg = i * P
                sp2 = psS.tile([P, 2, 512], F32, tag="s")
                for h in range(2):
                    hp = slice(h * 64, (h + 1) * 64)
                    nc.tensor.matmul(
                        sp2[:, h, col0:512],
                        kt_sbs[b][hp, sg:sg + P],
                        qt_sbs[b][hp, t0 + col0:t0 + 512],
                        start=True, stop=(delta < 0),
                    )
                    if delta >= 0:  # diagonal tile: add causal triangle on PE
                        nc.tensor.matmul(
                            sp2[:, h, col0:col0 + P], idn_sb[:], tri_sb[:],
                            start=False, stop=True,
                        )
                pt2 = ppool.tile([P, 2, 512], io_dt, tag="pt")
                nc.scalar.activation(
                    pt2[:, :, col0:512], sp2[:, :, col0:512], AF.Exp,
                )
                for h in range(2):
                    nc.tensor.matmul(
                        cps[h][:, col0:512],
                        v_sbs[b][:, i, h * 65:(h + 1) * 65],
                        pt2[:, h, col0:512],
                        start=(i == 0), stop=(i == nv - 1),
                    )
            for h in range(2):
                hp = slice(h * 64, (h + 1) * 64)
                # denom row -> SBUF bf16, broadcast across partitions on the
                # PE (bf16 1 cyc/row), then reciprocal of the whole [64,512]
                # tile on DVE (cost is per-column, same as a 1-partition op).
                den = spool.tile([1, 512], io_dt, tag="den")
                nc.scalar.copy(den[:], cps[h][64:65, :])
                bps = psO.tile([P, 512], F32, tag="o")
                nc.tensor.matmul(
                    bps[0:64, :], ones_sb[:, 0:64], den[:],
                    start=True, stop=True,
                )
                bc = spool.tile([64, 512], F32, tag="bc")
                nc.vector.reciprocal_approx_fast(bc[:], bps[0:64, :])
                nc.vector.tensor_mul(cn[hp, :], cps[h][0:64, :], bc[:])
            def flush_outproj():
                for rt4 in range(4):
                    r0 = b * S + t0 + rt4 * P
                    for fc in range(2):
                        ops = psO.tile([P, 512], F32, tag="o")
                        nc.tensor.matmul(
                            ops[:],
                            cn[:, rt4 * P:(rt4 + 1) * P],
                            ow_sb[:, fc * 512:(fc + 1) * 512],
                            start=True, stop=True,
                        )
                        ot = opool.tile([P, 512], io_dt, tag="ot")
                        # split PSUM->SBUF staging between ACT and DVE
                        if fc == 0:
                            nc.scalar.copy(ot[:], ops[:])
                        else:
                            nc.vector.tensor_copy(ot[:], ops[:])
                        nc.sync.dma_start(
                            out_d[r0:r0 + P, fc * 512:(fc + 1) * 512], ot[:]
                        )
            pending.append(flush_outproj)

        for _rep in range(rep):
            # batch-pipelined issue: proj(b0); attn(b) interleaved with proj(b+1)
            for ch in range(NCH):
                proj_chunk(0, ch)
            for b in range(B):
                for j in range(TJ):
                    if b + 1 < B:
                        proj_chunk(b + 1, j)
                    attn_chunk(b, j)
            while pending:
                pending.pop()()
    nc.compile()
    return nc


def make_core_inputs(x, key_padding_mask, Wqkv_w, Wqkv_b, out_w, B=4, S=2048,
                     np_io=None):
    """Host-side shard prep. Returns list of in_maps per core."""
    import ml_dtypes
    if np_io is None:
        np_io = ml_dtypes.bfloat16
    E = 1024
    P = 128
    NT = S // P
    x = np.asarray(x, np.float32)
    mask = np.asarray(key_padding_mask)
    Wqkv_w = np.asarray(Wqkv_w, np.float32)
    Wqkv_b = np.asarray(Wqkv_b, np.float32)
    out_w = np.asarray(out_w, np.float32)

    xT = np.ascontiguousarray(x.reshape(B * S, E).T).astype(np_io)
    m01 = mask.astype(np.float32)  # 1 valid / 0 padded
    msk_t = np.ascontiguousarray(m01.reshape(B * NT, P).T)  # [128, B*NT]
    r = np.arange(P)
    tri = np.where(r[:, None] > r[None, :], NEG, 0.0).astype(np_io)
    idn = np.eye(P, dtype=np.float32).astype(np_io)
    scale = 1.0 / np.sqrt(64.0)

    in_maps = []
    for c in range(N_CORES):
        hA, hB = 2 * c, 2 * c + 1
        sel = np.r_[hA * 64:(hA + 1) * 64, hB * 64:(hB + 1) * 64]
        wq = np.ascontiguousarray(Wqkv_w[sel].T).astype(np_io)
        wk = np.ascontiguousarray((Wqkv_w[E + sel] * scale).T).astype(np_io)
        wv = np.ascontiguousarray(Wqkv_w[2 * E + sel].T).astype(np_io)
        bq = np.ascontiguousarray(Wqkv_b[sel][:, None]).astype(np.float32)
        bv = np.ascontiguousarray(Wqkv_b[2 * E + sel][:, None]).astype(np.float32)
        ow = np.ascontiguousarray(out_w[:, sel].T).astype(np_io)
        in_maps.append({
            "xT": xT, "wq": wq, "wk": wk, "wv": wv,
            "bq": bq, "bv": bv, "ow": ow, "msk": msk_t,
            "tri": tri, "idn": idn,
        })
    return in_maps


_NC_CACHE = {}


def _get_nc(B=4, S=2048, io_dt=BF16, rep=1):
    key = (B, S, io_dt, rep)
    if key not in _NC_CACHE:
        _NC_CACHE[key] = build_program(B, S, io_dt, rep=rep)
    return _NC_CACHE[key]


def run_full(inputs, trace=False, tmpdir=None, io_dt=BF16, np_io=None):
    from concourse.bass_utils import run_bass_kernel_spmd

    B, S, E = 4, 2048, 1024
    nc = _get_nc(B, S, io_dt)
    in_maps = make_core_inputs(
        inputs["x"], inputs["key_padding_mask"], inputs["Wqkv_w"],
        inputs["Wqkv_b"], inputs["out_w"], B, S, np_io=np_io,
    )
    res = run_bass_kernel_spmd(
        nc, in_maps, list(range(N_CORES)), trace=trace, tmpdir=tmpdir,
    )
    acc = res.results[0]["outp"].astype(np.float32)
    for c in range(1, N_CORES):
        acc = acc + res.results[c]["outp"].astype(np.float32)
    out = acc + np.asarray(inputs["out_b"], np.float32)[None, :]
    return out.reshape(B, S, E), res


def kernel(**inputs) -> np.ndarray:
    out, _ = run_full(inputs)
    return out


# revision 25
# speedup vs baseline: 1.0104x; 1.0104x over previous
"""Trainium2 Bass kernel for causal MHA block (b=4, s=2048, E=1024, 16 heads).

Sharding: tensor-parallel over heads — 2 heads per core across 8 cores.
Each core computes Q^T/K^T (transposed layout, head-packed), V (natural
layout, ones-augmented), block-causal attention with softmax denominators
obtained for free from the ones column, and a partial out-projection over
its 128 embedding dims. Host sums the 8 partials and adds out_b.

bf16 fast path (matmul = 1 cycle/row vs fp32's 4):
  - all matmul operands bf16; PSUM accumulation stays fp32.
  - key-padding mask folded into V: masked keys get v rows and the
    ones-augmentation column zeroed (per-partition tensor_scalar_mul), so
    exp needs no bias and both heads' score tiles share one ACT
    instruction ([128, 2, W] AP over a 2-bank PSUM tile).
  - causal triangle added on the PE: identity @ tri accumulated into the
    diagonal 128-col subregion of the score tile.
  - 1/denom broadcast across partitions via gpsimd partition_broadcast.
  - V projection emits both heads + ones columns in one 130-wide matmul
    group (zero weight column + bias 1.0 produces the ones column).
  - out-proj PSUM->SBUF staging split between Pool and DVE engines.
  - phase A (projections) issue-interleaved with attention one batch
    ahead so ACT/DVE start early and drain before the PE tail.
"""

import sys
from contextlib import ExitStack

import numpy as np

sys.path.insert(0, "/opt/trn_rl_repo")

import concourse.bass as bass  # noqa: E402
import concourse.tile as tile  # noqa: E402
from concourse import bacc  # noqa: E402
from concourse import mybir  # noqa: E402

F32 = mybir.dt.float32
BF16 = mybir.dt.bfloat16
AF = mybir.ActivationFunctionType

NEG = -10000.0
N_CORES = 8


def build_program(B=4, S=2048, io_dt=BF16, rep=1):
    """Build the single-core SPMD program. Returns nc."""
    P = 128
    E = 1024
    ET = E // P            # 8 E-tiles
    RC = 512               # row chunk for projections
    NCH = S // RC          # proj chunks per batch (4)
    NT = S // P            # s-tiles per batch (16)
    TJ = S // 512          # query chunks of 512 per batch (4)
    ROWS = B * S

    nc = bacc.Bacc("TRN2", target_bir_lowering=False, debug=False)

    xT_d = nc.declare_dram_parameter("xT", [E, ROWS], io_dt, isOutput=False)
    wq_d = nc.declare_dram_parameter("wq", [E, P], io_dt, isOutput=False)
    wk_d = nc.declare_dram_parameter("wk", [E, P], io_dt, isOutput=False)
    wv_d = nc.declare_dram_parameter("wv", [E, P], io_dt, isOutput=False)
    bq_d = nc.declare_dram_parameter("bq", [P, 1], F32, isOutput=False)
    bv_d = nc.declare_dram_parameter("bv", [P, 1], F32, isOutput=False)
    ow_d = nc.declare_dram_parameter("ow", [P, E], io_dt, isOutput=False)
    msk_d = nc.declare_dram_parameter("msk", [P, B * NT], F32, isOutput=False)
    tri_d = nc.declare_dram_parameter("tri", [P, P], io_dt, isOutput=False)
    idn_d = nc.declare_dram_parameter("idn", [P, P], io_dt, isOutput=False)
    out_d = nc.declare_dram_parameter("outp", [ROWS, E], io_dt, isOutput=True)

    with ExitStack() as ctx:
        tc = ctx.enter_context(tile.TileContext(nc))
        const = ctx.enter_context(tc.tile_pool(name="const", bufs=1))

        wq_sb = const.tile([P, ET, P], io_dt, tag="wq")
        wk_sb = const.tile([P, ET, P], io_dt, tag="wk")
        wv_sb = const.tile([P, ET, P], io_dt, tag="wv")
        # spread big const loads across engine DMA queues so they transfer
        # in parallel instead of serializing on one queue
        nc.gpsimd.dma_start(wq_sb[:], wq_d[:].rearrange("(et p) f -> p et f", p=P))
        nc.scalar.dma_start(wk_sb[:], wk_d[:].rearrange("(et p) f -> p et f", p=P))
        nc.sync.dma_start(wv_sb[:], wv_d[:].rearrange("(et p) f -> p et f", p=P))
        ow_sb = const.tile([P, E], io_dt, tag="ow")
        nc.scalar.dma_start(ow_sb[:], ow_d[:])
        bq_sb = const.tile([P, 1], F32, tag="bq")
        nc.gpsimd.dma_start(bq_sb[:], bq_d[:])
        bv_sb = const.tile([P, 1], F32, tag="bv")
        nc.gpsimd.dma_start(bv_sb[:], bv_d[:])
        msk_sb = const.tile([P, B * NT], F32, tag="msk")
        nc.gpsimd.dma_start(msk_sb[:], msk_d[:])
        tri_sb = const.tile([P, P], io_dt, tag="tri")
        nc.gpsimd.dma_start(tri_sb[:], tri_d[:])
        idn_sb = const.tile([P, P], io_dt, tag="idn")
        nc.gpsimd.dma_start(idn_sb[:], idn_d[:])
        ones_sb = const.tile([1, P], io_dt, tag="ones")
        nc.any.memset(ones_sb[:], 1.0)

        # per-batch projection outputs
        qt_sbs = [const.tile([P, S], io_dt, tag=f"qt{b}", name=f"qt{b}") for b in range(B)]
        kt_sbs = [const.tile([P, S], io_dt, tag=f"kt{b}", name=f"kt{b}") for b in range(B)]
        v_sbs = [const.tile([P, NT, 130], io_dt, tag=f"v{b}", name=f"v{b}") for b in range(B)]
        # ones-augmentation columns hold the key-padding mask (1 valid / 0
        # padded) so denominators count only valid keys
        for b in range(B):
            nc.gpsimd.dma_start(v_sbs[b][:, :, 64:65], msk_d[:, b * NT:(b + 1) * NT])
            nc.gpsimd.dma_start(v_sbs[b][:, :, 129:130], msk_d[:, b * NT:(b + 1) * NT])

        xpool = ctx.enter_context(tc.tile_pool(name="xp", bufs=2))
        ppool = ctx.enter_context(tc.tile_pool(name="pt", bufs=4))
        cpool = ctx.enter_context(tc.tile_pool(name="cn", bufs=2))
        spool = ctx.enter_context(tc.tile_pool(name="sm", bufs=2))
        opool = ctx.enter_context(tc.tile_pool(name="ot", bufs=4))
        # PSUM: "s" = 2-bank [128,2,512] tiles shared by phase A groups and
        # attention score pairs (2 bufs = 4 banks); "c" ctx accum (2 banks);
        # "o" out-proj (2 banks).  Total 8 banks.
        psS = ctx.enter_context(tc.tile_pool(name="psS", bufs=2, space="PSUM"))
        psC = ctx.enter_context(tc.tile_pool(name="psC", bufs=2, space="PSUM"))
        psO = ctx.enter_context(tc.tile_pool(name="psO", bufs=2, space="PSUM"))

        def proj_chunk(b, ch):
            """Project rows [b*S + ch*RC, +RC) -> qt/kt/v for batch b."""
            r0 = b * S + ch * RC
            rsb = slice(ch * RC, (ch + 1) * RC)
            xt = xpool.tile([P, ET, RC], io_dt, tag="xt")
            for et in range(ET):
                nc.sync.dma_start(xt[:, et], xT_d[et * P:(et + 1) * P, r0:r0 + RC])
            for w_sb, dst, bias in ((wq_sb, qt_sbs[b], bq_sb), (wk_sb, kt_sbs[b], None)):
                ps = psS.tile([P, 2, RC], F32, tag="s")
                for et in range(ET):
                    nc.tensor.matmul(
                        ps[:, 0], w_sb[:, et, 0:P], xt[:, et],
                        start=(et == 0), stop=(et == ET - 1),
                    )
                if bias is not None:
                    nc.vector.tensor_scalar_add(dst[:, rsb], ps[:, 0], bias[:])
                else:
                    nc.vector.tensor_copy(dst[:, rsb], ps[:, 0])
            # V computed transposed like Q/K (few big matmuls), bias added on
            # the per-partition copy, then PE-transposed into [keys, dims]
            # layout with the key-padding mask folded in on eviction.
            psv = psS.tile([P, 2, RC], F32, tag="s")
            for et in range(ET):
                nc.tensor.matmul(
                    psv[:, 0], wv_sb[:, et], xt[:, et],
                    start=(et == 0), stop=(et == ET - 1),
                )
            vt = xpool.tile([P, RC], io_dt, tag="vt")
            nc.vector.tensor_scalar_add(vt[:], psv[:, 0], bv_sb[:])
            for rt4 in range(RC // P):
                rt = ch * (RC // P) + rt4
                trp = psO.tile([P, P], io_dt, tag="o")
                nc.tensor.transpose(trp[:], vt[:, rt4 * P:(rt4 + 1) * P], idn_sb[:])
                msc = msk_sb[:, b * NT + rt:b * NT + rt + 1]
                nc.vector.tensor_scalar_mul(v_sbs[b][:, rt, 0:64], trp[:, 0:64], msc)
                nc.vector.tensor_scalar_mul(v_sbs[b][:, rt, 65:129], trp[:, 64:128], msc)

        pending = []  # deferred out-proj of the previous chunk

        def attn_chunk(b, j):
            """Attention for query chunk j of batch b; out-proj deferred into
            the next chunk's score loop so the PE never head-of-line-blocks
            on the normalization chain."""
            t0 = j * 512
            nv = 4 * j + 4
            cn = cpool.tile([P, 512], io_dt, tag="cn")
            cps = [psC.tile([65, 512], F32, tag="c", name=f"cps{h}") for h in range(2)]
            for i in range(nv):
                if i == 2 and pending:
                    pending.pop()()
                delta = i * P - t0
                col0 = max(0, delta)
                sg = i * P
                sp2 = psS.tile([P, 2, 512], F32, tag="s")
                for h in range(2):
                    hp = slice(h * 64, (h + 1) * 64)
                    nc.tensor.matmul(
                        sp2[:, h, col0:512],
                        kt_sbs[b][hp, sg:sg + P],
                        qt_sbs[b][hp, t0 + col0:t0 + 512],
                        start=True, stop=(delta < 0),
                    )
                    if delta >= 0:  # diagonal tile: add causal triangle on PE
                        nc.tensor.matmul(
                            sp2[:, h, col0:col0 + P], idn_sb[:], tri_sb[:],
                            start=False, stop=True,
                        )
                pt2 = ppool.tile([P, 2, 512], io_dt, tag="pt")
                nc.scalar.activation(
                    pt2[:, :, col0:512], sp2[:, :, col0:512], AF.Exp,
                )
                for h in range(2):
                    nc.tensor.matmul(
                        cps[h][:, col0:512],
                        v_sbs[b][:, i, h * 65:(h + 1) * 65],
                        pt2[:, h, col0:512],
                        start=(i == 0), stop=(i == nv - 1),
                    )
            for h in range(2):
                hp = slice(h * 64, (h + 1) * 64)
                # denom row -> SBUF bf16, broadcast across partitions on the
                # PE (bf16 1 cyc/row), then reciprocal of the whole [64,512]
                # tile on DVE (cost is per-column, same as a 1-partition op).
                den = spool.tile([1, 512], io_dt, tag="den")
                nc.scalar.copy(den[:], cps[h][64:65, :])
                bps = psO.tile([P, 512], F32, tag="o")
                nc.tensor.matmul(
                    bps[0:64, :], ones_sb[:, 0:64], den[:],
                    start=True, stop=True,
                )
                bc = spool.tile([64, 512], F32, tag="bc")
                nc.vector.reciprocal_approx_fast(bc[:], bps[0:64, :])
                nc.vector.tensor_mul(cn[hp, :], cps[h][0:64, :], bc[:])
            def flush_outproj():
                for rt4 in range(4):
                    r0 = b * S + t0 + rt4 * P
                    for fc in range(2):
                        ops = psO.tile([P, 512], F32, tag="o")
                        nc.tensor.matmul(
                            ops[:],
                            cn[:, rt4 * P:(rt4 + 1) * P],
                            ow_sb[:, fc * 512:(fc + 1) * 512],
                            start=True, stop=True,
                        )
                        ot = opool.tile([P, 512], io_dt, tag="ot")
                        # split PSUM->SBUF staging between ACT and DVE
                        if fc == 0:
                            nc.scalar.copy(ot[:], ops[:])
                        else:
                            nc.vector.tensor_copy(ot[:], ops[:])
                        nc.sync.dma_start(
                            out_d[r0:r0 + P, fc * 512:(fc + 1) * 512], ot[:]
                        )
            pending.append(flush_outproj)

        for _rep in range(rep):
            # batch-pipelined issue: proj(b0); attn(b) interleaved with proj(b+1)
            for ch in range(NCH):
                proj_chunk(0, ch)
            for b in range(B):
                for j in range(TJ):
                    if b + 1 < B:
                        proj_chunk(b + 1, j)
                    attn_chunk(b, j)
            while pending:
                pending.pop()()
    nc.compile()
    return nc


def make_core_inputs(x, key_padding_mask, Wqkv_w, Wqkv_b, out_w, B=4, S=2048,
                     np_io=None):
    """Host-side shard prep. Returns list of in_maps per core."""
    import ml_dtypes
    if np_io is None:
        np_io = ml_dtypes.bfloat16
    E = 1024
    P = 128
    NT = S // P
    x = np.asarray(x, np.float32)
    mask = np.asarray(key_padding_mask)
    Wqkv_w = np.asarray(Wqkv_w, np.float32)
    Wqkv_b = np.asarray(Wqkv_b, np.float32)
    out_w = np.asarray(out_w, np.float32)

    xT = np.ascontiguousarray(x.reshape(B * S, E).T).astype(np_io)
    m01 = mask.astype(np.float32)  # 1 valid / 0 padded
    msk_t = np.ascontiguousarray(m01.reshape(B * NT, P).T)  # [128, B*NT]
    r = np.arange(P)
    tri = np.where(r[:, None] > r[None, :], NEG, 0.0).astype(np_io)
    idn = np.eye(P, dtype=np.float32).astype(np_io)
    scale = 1.0 / np.sqrt(64.0)

    in_maps = []
    for c in range(N_CORES):
        hA, hB = 2 * c, 2 * c + 1
        sel = np.r_[hA * 64:(hA + 1) * 64, hB * 64:(hB + 1) * 64]
        wq = np.ascontiguousarray(Wqkv_w[sel].T).astype(np_io)
        wk = np.ascontiguousarray((Wqkv_w[E + sel] * scale).T).astype(np_io)
        wv = np.ascontiguousarray(Wqkv_w[2 * E + sel].T).astype(np_io)
        bq = np.ascontiguousarray(Wqkv_b[sel][:, None]).astype(np.float32)
        bv = np.ascontiguousarray(Wqkv_b[2 * E + sel][:, None]).astype(np.float32)
        ow = np.ascontiguousarray(out_w[:, sel].T).astype(np_io)
        in_maps.append({
            "xT": xT, "wq": wq, "wk": wk, "wv": wv,
            "bq": bq, "bv": bv, "ow": ow, "msk": msk_t,
            "tri": tri, "idn": idn,
        })
    return in_maps


_NC_CACHE = {}


def _get_nc(B=4, S=2048, io_dt=BF16, rep=1):
    key = (B, S, io_dt, rep)
    if key not in _NC_CACHE:
        _NC_CACHE[key] = build_program(B, S, io_dt, rep=rep)
    return _NC_CACHE[key]


def run_full(inputs, trace=False, tmpdir=None, io_dt=BF16, np_io=None):
    from concourse.bass_utils import run_bass_kernel_spmd

    B, S, E = 4, 2048, 1024
    nc = _get_nc(B, S, io_dt)
    in_maps = make_core_inputs(
        inputs["x"], inputs["key_padding_mask"], inputs["Wqkv_w"],
        inputs["Wqkv_b"], inputs["out_w"], B, S, np_io=np_io,
    )
    res = run_bass_kernel_spmd(
        nc, in_maps, list(range(N_CORES)), trace=trace, tmpdir=tmpdir,
    )
    acc = res.results[0]["outp"].astype(np.float32)
    for c in range(1, N_CORES):
        acc = acc + res.results[c]["outp"].astype(np.float32)
    out = acc + np.asarray(inputs["out_b"], np.float32)[None, :]
    return out.reshape(B, S, E), res


def kernel(**inputs) -> np.ndarray:
    out, _ = run_full(inputs)
    return out


# revision 26
# speedup vs baseline: 1.0522x; 1.0413x over previous
"""Trainium2 Bass kernel for causal MHA block (b=4, s=2048, E=1024, 16 heads).

Sharding: tensor-parallel over heads — 2 heads per core across 8 cores.
Each core computes Q^T/K^T (transposed layout, head-packed), V (natural
layout, ones-augmented), block-causal attention with softmax denominators
obtained for free from the ones column, and a partial out-projection over
its 128 embedding dims. Host sums the 8 partials and adds out_b.

bf16 fast path (matmul = 1 cycle/row vs fp32's 4):
  - all matmul operands bf16; PSUM accumulation stays fp32.
  - key-padding mask folded into V: masked keys get v rows and the
    ones-augmentation column zeroed (per-partition tensor_scalar_mul), so
    exp needs no bias and both heads' score tiles share one ACT
    instruction ([128, 2, W] AP over a 2-bank PSUM tile).
  - causal triangle added on the PE: identity @ tri accumulated into the
    diagonal 128-col subregion of the score tile.
  - 1/denom broadcast across partitions via gpsimd partition_broadcast.
  - V projection emits both heads + ones columns in one 130-wide matmul
    group (zero weight column + bias 1.0 produces the ones column).
  - out-proj PSUM->SBUF staging split between Pool and DVE engines.
  - phase A (projections) issue-interleaved with attention one batch
    ahead so ACT/DVE start early and drain before the PE tail.
"""

import sys
from contextlib import ExitStack

import numpy as np

sys.path.insert(0, "/opt/trn_rl_repo")

import concourse.bass as bass  # noqa: E402
import concourse.tile as tile  # noqa: E402
from concourse import bacc  # noqa: E402
from concourse import mybir  # noqa: E402

F32 = mybir.dt.float32
BF16 = mybir.dt.bfloat16
AF = mybir.ActivationFunctionType

NEG = -10000.0
N_CORES = 8


def build_program(B=4, S=2048, io_dt=BF16, rep=1):
    """Build the single-core SPMD program. Returns nc."""
    P = 128
    E = 1024
    ET = E // P            # 8 E-tiles
    RC = 512               # row chunk for projections
    NCH = S // RC          # proj chunks per batch (4)
    NT = S // P            # s-tiles per batch (16)
    TJ = S // 512          # query chunks of 512 per batch (4)
    ROWS = B * S

    nc = bacc.Bacc("TRN2", target_bir_lowering=False, debug=False)

    xT_d = nc.declare_dram_parameter("xT", [E, ROWS], io_dt, isOutput=False)
    wq_d = nc.declare_dram_parameter("wq", [E, P], io_dt, isOutput=False)
    wk_d = nc.declare_dram_parameter("wk", [E, P], io_dt, isOutput=False)
    wv_d = nc.declare_dram_parameter("wv", [E, P], io_dt, isOutput=False)
    bq_d = nc.declare_dram_parameter("bq", [P, 1], F32, isOutput=False)
    bv_d = nc.declare_dram_parameter("bv", [P, 1], F32, isOutput=False)
    ow_d = nc.declare_dram_parameter("ow", [P, E], io_dt, isOutput=False)
    msk_d = nc.declare_dram_parameter("msk", [P, B * NT], F32, isOutput=False)
    tri_d = nc.declare_dram_parameter("tri", [P, P], io_dt, isOutput=False)
    idn_d = nc.declare_dram_parameter("idn", [P, P], io_dt, isOutput=False)
    out_d = nc.declare_dram_parameter("outp", [ROWS, E], io_dt, isOutput=True)

    with ExitStack() as ctx:
        tc = ctx.enter_context(tile.TileContext(nc))
        const = ctx.enter_context(tc.tile_pool(name="const", bufs=1))

        # one tile per 128-row weight slice: whole-tile DMA writes (clean
        # deps) and the first projection matmuls unblock after 32KB
        wq_sb = [const.tile([P, P], io_dt, tag=f"wq{et}", name=f"wq{et}")
                 for et in range(ET)]
        wk_sb = [const.tile([P, P], io_dt, tag=f"wk{et}", name=f"wk{et}")
                 for et in range(ET)]
        wv_sb = [const.tile([P, P], io_dt, tag=f"wv{et}", name=f"wv{et}")
                 for et in range(ET)]
        for et in range(ET):
            esl = slice(et * P, (et + 1) * P)
            nc.gpsimd.dma_start(wq_sb[et][:], wq_d[esl, :])
            nc.scalar.dma_start(wk_sb[et][:], wk_d[esl, :])
            (nc.gpsimd if et % 2 else nc.scalar).dma_start(wv_sb[et][:], wv_d[esl, :])
        ow_sb = const.tile([P, E], io_dt, tag="ow")
        nc.scalar.dma_start(ow_sb[:], ow_d[:])
        bq_sb = const.tile([P, 1], F32, tag="bq")
        nc.gpsimd.dma_start(bq_sb[:], bq_d[:])
        bv_sb = const.tile([P, 1], F32, tag="bv")
        nc.gpsimd.dma_start(bv_sb[:], bv_d[:])
        msk_sb = const.tile([P, B * NT], F32, tag="msk")
        nc.gpsimd.dma_start(msk_sb[:], msk_d[:])
        tri_sb = const.tile([P, P], io_dt, tag="tri")
        nc.gpsimd.dma_start(tri_sb[:], tri_d[:])
        idn_sb = const.tile([P, P], io_dt, tag="idn")
        nc.gpsimd.dma_start(idn_sb[:], idn_d[:])
        ones_sb = const.tile([1, P], io_dt, tag="ones")
        nc.any.memset(ones_sb[:], 1.0)

        # per-batch projection outputs
        qt_sbs = [const.tile([P, S], io_dt, tag=f"qt{b}", name=f"qt{b}") for b in range(B)]
        kt_sbs = [const.tile([P, S], io_dt, tag=f"kt{b}", name=f"kt{b}") for b in range(B)]
        v_sbs = [const.tile([P, NT, 130], io_dt, tag=f"v{b}", name=f"v{b}") for b in range(B)]
        # ones-augmentation columns hold the key-padding mask (1 valid / 0
        # padded) so denominators count only valid keys
        for b in range(B):
            nc.gpsimd.dma_start(v_sbs[b][:, :, 64:65], msk_d[:, b * NT:(b + 1) * NT])
            nc.gpsimd.dma_start(v_sbs[b][:, :, 129:130], msk_d[:, b * NT:(b + 1) * NT])

        xpool = ctx.enter_context(tc.tile_pool(name="xp", bufs=2))
        ppool = ctx.enter_context(tc.tile_pool(name="pt", bufs=4))
        cpool = ctx.enter_context(tc.tile_pool(name="cn", bufs=2))
        spool = ctx.enter_context(tc.tile_pool(name="sm", bufs=2))
        opool = ctx.enter_context(tc.tile_pool(name="ot", bufs=4))
        # PSUM: "s" = 2-bank [128,2,512] tiles shared by phase A groups and
        # attention score pairs (2 bufs = 4 banks); "c" ctx accum (2 banks);
        # "o" out-proj (2 banks).  Total 8 banks.
        psS = ctx.enter_context(tc.tile_pool(name="psS", bufs=2, space="PSUM"))
        psC = ctx.enter_context(tc.tile_pool(name="psC", bufs=2, space="PSUM"))
        psO = ctx.enter_context(tc.tile_pool(name="psO", bufs=2, space="PSUM"))

        def proj_chunk(b, ch):
            """Project rows [b*S + ch*RC, +RC) -> qt/kt/v for batch b."""
            r0 = b * S + ch * RC
            rsb = slice(ch * RC, (ch + 1) * RC)
            xt = xpool.tile([P, ET, RC], io_dt, tag="xt")
            for et in range(ET):
                nc.sync.dma_start(xt[:, et], xT_d[et * P:(et + 1) * P, r0:r0 + RC])
            for w_sb, dst, bias in ((wq_sb, qt_sbs[b], bq_sb), (wk_sb, kt_sbs[b], None)):
                ps = psS.tile([P, 2, RC], F32, tag="s")
                for et in range(ET):
                    nc.tensor.matmul(
                        ps[:, 0], w_sb[et][:], xt[:, et],
                        start=(et == 0), stop=(et == ET - 1),
                    )
                if bias is not None:
                    nc.vector.tensor_scalar_add(dst[:, rsb], ps[:, 0], bias[:])
                else:
                    nc.vector.tensor_copy(dst[:, rsb], ps[:, 0])
            # V computed transposed like Q/K (few big matmuls), bias added on
            # the per-partition copy, then PE-transposed into [keys, dims]
            # layout with the key-padding mask folded in on eviction.
            psv = psS.tile([P, 2, RC], F32, tag="s")
            for et in range(ET):
                nc.tensor.matmul(
                    psv[:, 0], wv_sb[et][:], xt[:, et],
                    start=(et == 0), stop=(et == ET - 1),
                )
            vt = xpool.tile([P, RC], io_dt, tag="vt")
            nc.vector.tensor_scalar_add(vt[:], psv[:, 0], bv_sb[:])
            for rt4 in range(RC // P):
                rt = ch * (RC // P) + rt4
                trp = psO.tile([P, P], io_dt, tag="o")
                nc.tensor.transpose(trp[:], vt[:, rt4 * P:(rt4 + 1) * P], idn_sb[:])
                msc = msk_sb[:, b * NT + rt:b * NT + rt + 1]
                nc.vector.tensor_scalar_mul(v_sbs[b][:, rt, 0:64], trp[:, 0:64], msc)
                nc.vector.tensor_scalar_mul(v_sbs[b][:, rt, 65:129], trp[:, 64:128], msc)

        def attn_chunk(b, j):
            """Attention + out-proj for query chunk j of batch b."""
            t0 = j * 512
            nv = 4 * j + 4
            cn = cpool.tile([P, 512], io_dt, tag="cn")
            cps = [psC.tile([65, 512], F32, tag="c", name=f"cps{h}") for h in range(2)]
            for i in range(nv):
                delta = i * P - t0
                col0 = max(0, delta)
                sg = i * P
                sp2 = psS.tile([P, 2, 512], F32, tag="s")
                for h in range(2):
                    hp = slice(h * 64, (h + 1) * 64)
                    nc.tensor.matmul(
                        sp2[:, h, col0:512],
                        kt_sbs[b][hp, sg:sg + P],
                        qt_sbs[b][hp, t0 + col0:t0 + 512],
                        start=True, stop=(delta < 0),
                    )
                    if delta >= 0:  # diagonal tile: add causal triangle on PE
                        nc.tensor.matmul(
                            sp2[:, h, col0:col0 + P], idn_sb[:], tri_sb[:],
                            start=False, stop=True,
                        )
                pt2 = ppool.tile([P, 2, 512], io_dt, tag="pt")
                nc.scalar.activation(
                    pt2[:, :, col0:512], sp2[:, :, col0:512], AF.Exp,
                )
                for h in range(2):
                    nc.tensor.matmul(
                        cps[h][:, col0:512],
                        v_sbs[b][:, i, h * 65:(h + 1) * 65],
                        pt2[:, h, col0:512],
                        start=(i == 0), stop=(i == nv - 1),
                    )
            for h in range(2):
                hp = slice(h * 64, (h + 1) * 64)
                # denom row -> SBUF bf16, broadcast across partitions on the
                # PE (bf16 1 cyc/row), then reciprocal of the whole [64,512]
                # tile on DVE (cost is per-column, same as a 1-partition op).
                den = spool.tile([1, 512], io_dt, tag="den")
                nc.vector.tensor_copy(den[:], cps[h][64:65, :])
                bps = psO.tile([P, 512], F32, tag="o")
                nc.tensor.matmul(
                    bps[0:64, :], ones_sb[:, 0:64], den[:],
                    start=True, stop=True,
                )
                bc = spool.tile([64, 512], F32, tag="bc")
                nc.vector.reciprocal_approx_fast(bc[:], bps[0:64, :])
                nc.vector.tensor_mul(cn[hp, :], cps[h][0:64, :], bc[:])
            for rt4 in range(4):
                r0 = b * S + t0 + rt4 * P
                for fc in range(2):
                    ops = psO.tile([P, 512], F32, tag="o")
                    nc.tensor.matmul(
                        ops[:],
                        cn[:, rt4 * P:(rt4 + 1) * P],
                        ow_sb[:, fc * 512:(fc + 1) * 512],
                        start=True, stop=True,
                    )
                    ot = opool.tile([P, 512], io_dt, tag="ot")
                    nc.vector.tensor_copy(ot[:], ops[:])
                    nc.sync.dma_start(
                        out_d[r0:r0 + P, fc * 512:(fc + 1) * 512], ot[:]
                    )

        for _rep in range(rep):
            # batch-pipelined issue: proj(b0); attn(b) interleaved with proj(b+1)
            for ch in range(NCH):
                proj_chunk(0, ch)
            for b in range(B):
                for j in range(TJ):
                    if b + 1 < B:
                        proj_chunk(b + 1, j)
                    attn_chunk(b, j)
    nc.compile()
    return nc


def make_core_inputs(x, key_padding_mask, Wqkv_w, Wqkv_b, out_w, B=4, S=2048,
                     np_io=None):
    """Host-side shard prep. Returns list of in_maps per core."""
    import ml_dtypes
    if np_io is None:
        np_io = ml_dtypes.bfloat16
    E = 1024
    P = 128
    NT = S // P
    x = np.asarray(x, np.float32)
    mask = np.asarray(key_padding_mask)
    Wqkv_w = np.asarray(Wqkv_w, np.float32)
    Wqkv_b = np.asarray(Wqkv_b, np.float32)
    out_w = np.asarray(out_w, np.float32)

    xT = np.ascontiguousarray(x.reshape(B * S, E).T).astype(np_io)
    m01 = mask.astype(np.float32)  # 1 valid / 0 padded
    msk_t = np.ascontiguousarray(m01.reshape(B * NT, P).T)  # [128, B*NT]
    r = np.arange(P)
    tri = np.where(r[:, None] > r[None, :], NEG, 0.0).astype(np_io)
    idn = np.eye(P, dtype=np.float32).astype(np_io)
    scale = 1.0 / np.sqrt(64.0)

    in_maps = []
    for c in range(N_CORES):
        hA, hB = 2 * c, 2 * c + 1
        sel = np.r_[hA * 64:(hA + 1) * 64, hB * 64:(hB + 1) * 64]
        wq = np.ascontiguousarray(Wqkv_w[sel].T).astype(np_io)
        wk = np.ascontiguousarray((Wqkv_w[E + sel] * scale).T).astype(np_io)
        wv = np.ascontiguousarray(Wqkv_w[2 * E + sel].T).astype(np_io)
        bq = np.ascontiguousarray(Wqkv_b[sel][:, None]).astype(np.float32)
        bv = np.ascontiguousarray(Wqkv_b[2 * E + sel][:, None]).astype(np.float32)
        ow = np.ascontiguousarray(out_w[:, sel].T).astype(np_io)
        in_maps.append({
            "xT": xT, "wq": wq, "wk": wk, "wv": wv,
            "bq": bq, "bv": bv, "ow": ow, "msk": msk_t,
            "tri": tri, "idn": idn,
        })
    return in_maps


_NC_CACHE = {}


def _get_nc(B=4, S=2048, io_dt=BF16, rep=1):
    key = (B, S, io_dt, rep)
    if key not in _NC_CACHE:
        _NC_CACHE[key] = build_program(B, S, io_dt, rep=rep)
    return _NC_CACHE[key]


def run_full(inputs, trace=False, tmpdir=None, io_dt=BF16, np_io=None):
    from concourse.bass_utils import run_bass_kernel_spmd

    B, S, E = 4, 2048, 1024
    nc = _get_nc(B, S, io_dt)
    in_maps = make_core_inputs(
        inputs["x"], inputs["key_padding_mask"], inputs["Wqkv_w"],
        inputs["Wqkv_b"], inputs["out_w"], B, S, np_io=np_io,
    )
    res = run_bass_kernel_spmd(
        nc, in_maps, list(range(N_CORES)), trace=trace, tmpdir=tmpdir,
    )
    acc = res.results[0]["outp"].astype(np.float32)
    for c in range(1, N_CORES):
        acc = acc + res.results[c]["outp"].astype(np.float32)
    out = acc + np.asarray(inputs["out_b"], np.float32)[None, :]
    return out.reshape(B, S, E), res


def kernel(**inputs) -> np.ndarray:
    out, _ = run_full(inputs)
    return out
